# revision 5
# baseline (speedup 1.0000x reference)
"""Trainium2 Bass kernel for DualMem retrieval (exp-cosine kNN memory head), v2.

Contract: kernel(**inputs) takes FULL numpy inputs, returns FULL [1, C]
softmax output.  Class axis sharded over 8 NeuronCores.

Math (reference):
  q   = l2norm(img + mean_c(global_bias))                       [1, D]
  K/V = l2norm(mem + bias_{k,v}[c]) masked where row(mem)==0    [C,Mt,D]
  sim = exp(-beta*(1 - q.K))                                    [C,Mt]
  a   = l2norm(l2norm(sim @ V) + ffn_bias)                      [C, D]
  out = softmax(exp(ls) * a @ img)                              [1, C]

Structural changes vs the first-generation kernel (110.7us -> ~99us):
  - mean(global_bias) from a host-transposed d-major copy of gb reduced
    locally on DVE (bf16, 2x mode) -- replaces a 12.6us f32 HWDGE load +
    ~14us of f32 colsum matmuls on the PE; qhat is ready by ~11us.
  - mem/bias tables stored bf16 in DRAM (host cast): halves DMA bytes and
    allows the batches to stream on the SP HWDGE queue (casting DMAs are
    SWDGE-only), keeping GPSIMD free; all 33 rowblocks are buffered in
    SBUF so the DMA device never stalls on consumers.
  - fused dot-pass: bk/bv dots, q dot and the per-class constant gather
    ([||bk||^2, ||bv||^2, bk.qhat] via one fp8 one-hot matmul) accumulate
    into one [128,14] psum -> one evacuation copy per rowblock.
  - extraction does k/v lanes together on [128,g,2] tiles; rsqrt is a
    quadratic seed + 1 Newton step (tolerance is 2e-2); weight rowblocks
    are built on GPSIMD.
  - engine balance: transpose copybacks on DVE, row-square reductions
    rotated over ACT/GPSIMD (K_SQ/K_CB env knobs).
  - K_RDMA=1 (sim-only experiment; the GPSIMD remote-DMA ucode is not
    available in this runner's NEFF image) replaces both AllGather
    collectives (15us constant overhead each) with direct SBUF remote
    DMAs between the 8 cores: ~80us in the cost model.
"""

import os
import sys

sys.path.insert(0, "/opt/trn_rl_repo")

import numpy as np

import concourse.bass as bass
import concourse.mybir as mybir
import concourse.tile as tile
from concourse import bacc
from concourse.bass_utils import run_bass_kernel_spmd

F32 = mybir.dt.float32
BF16 = mybir.dt.bfloat16
FP8 = mybir.dt.float8e4
I32 = mybir.dt.int32
AF = mybir.ActivationFunctionType
ALU = mybir.AluOpType
AX = mybir.AxisListType

BETA = 5.5
N_CORES = 8
C, MT, D = 1000, 33, 1024
CPC = C // N_CORES          # classes per core
R = CPC * MT                # rows per core
NRB = (R + 127) // 128      # rowblocks per core
NCH = D // 128              # 128-wide d-chunks
QB = 4                      # rowblocks per mem DMA batch
NQB = (NRB + QB - 1) // QB
CP = CPC + 7                # padded class dim so [c0:c0+5] slices stay in range

GROUPS = tuple(int(x) for x in os.environ.get('K_GROUPS', '11,11,11').split(','))
GRP = max(GROUPS)
# per-rowblock square engine: 'd'=DVE  'a'=ACT   (pattern repeats)
SQ_PAT = os.environ.get('K_SQ', 'adaada')
# per-rowblock transpose-copyback engine: 'd'=DVE 'a'=ACT
CB_PAT = os.environ.get('K_CB', 'dddddd')
K_RDMA = int(os.environ.get('K_RDMA', '0'))
RSQ_ITERS = int(os.environ.get('K_RSQI', '1'))

# logical core -> physical nc index (TRN2 period-8 map)
NC_BASE = (0, 1, 2, 3, 6, 7, 4, 5)
RD_MASK = 0x10   # DMA engine 4: reaches both same-die and cross-die peers

RSQ_A = (0.05888337527349581, -3.735601567857182e-05, 1.02184149458168e-08)
RSQ_B = (1.6460793992359617, -0.7401760506078425, 0.1316746462210596)
MAGIC = 0x5f3759df


def _rb_info(rb):
    r0 = rb * 128
    nr = min(128, R - r0)
    c0 = r0 // MT
    return r0, nr, c0


def _host_constants():
    """Compile-time one-hot tensors from the class-major row layout
    (row r of a core = class r//MT, slot r%MT)."""
    ext = np.zeros((128, NRB, 10), np.float32)
    ind = np.zeros((CPC, NRB, 128), np.float32)
    cmask = np.zeros((128, NRB, CPC), np.float32)
    for rb in range(NRB):
        r0, nr, c0 = _rb_info(rb)
        for p in range(nr):
            c = (r0 + p) // MT
            jc = c - c0
            ext[p, rb, jc] = 1.0
            ext[p, rb, 5 + jc] = 1.0
            ind[c, rb, p] = 1.0
            cmask[p, rb, c] = 1.0
    ident = np.eye(128, dtype=np.float32)
    return {
        "ext": ext.reshape(128, NRB * 10),
        "ind": ind.reshape(CPC, NRB * 128),
        "cmask": cmask.reshape(128, NRB * CPC),
        "ident": ident,
    }


def _emit_rsqrt(nc, pool, out, x, coef, iters, tag):
    """out = x**-0.5 via quadratic seed (fitted range) + Newton steps."""
    c0, c1, c2 = coef
    shp, dt = list(x.shape), F32
    t = pool.tile(shp, dt, tag=tag + "t")
    nc.vector.tensor_scalar(t[:], x, c2, c1, op0=ALU.mult, op1=ALU.add)
    y = pool.tile(shp, dt, tag=tag + "y")
    nc.vector.scalar_tensor_tensor(y[:], t[:], 1.0, x, op0=ALU.mult, op1=ALU.mult)
    nc.vector.tensor_scalar(y[:], y[:], c0, None, op0=ALU.add)
    for _ in range(iters):
        a = pool.tile(shp, dt, tag=tag + "a")
        nc.vector.scalar_tensor_tensor(a[:], y[:], 1.0, y[:], op0=ALU.mult,
                                       op1=ALU.mult)
        nc.vector.scalar_tensor_tensor(a[:], a[:], -0.5, x, op0=ALU.mult,
                                       op1=ALU.mult)
        nc.vector.tensor_scalar(a[:], a[:], 1.5, None, op0=ALU.add)
        nc.vector.tensor_tensor(y[:], y[:], a[:], op=ALU.mult)
    nc.vector.tensor_copy(out, y[:])


def _emit_rsqrt_magic(nc, pool, out, x, iters, tag):
    """out = x**-0.5 via int bit-magic seed + Newton (any positive range)."""
    shp = list(x.shape)
    yi = pool.tile(shp, mybir.dt.int32, tag=tag + "i")
    nc.vector.tensor_scalar(yi[:], x.bitcast(mybir.dt.int32), 1, None,
                            op0=ALU.logical_shift_right)
    nc.vector.tensor_scalar(yi[:], yi[:], MAGIC, -1, op0=ALU.subtract,
                            op1=ALU.mult)
    y = yi[:].bitcast(F32)
    for _ in range(iters):
        a = pool.tile(shp, F32, tag=tag + "a")
        nc.vector.scalar_tensor_tensor(a[:], y, 1.0, y, op0=ALU.mult,
                                       op1=ALU.mult)
        nc.vector.scalar_tensor_tensor(a[:], a[:], -0.5, x, op0=ALU.mult,
                                       op1=ALU.mult)
        nc.vector.tensor_scalar(a[:], a[:], 1.5, None, op0=ALU.add)
        nc.vector.tensor_tensor(y, y, a[:], op=ALU.mult)
    nc.vector.tensor_copy(out, y)


def build_nc():
    nc = bacc.Bacc("TRN2", target_bir_lowering=False, debug=False,
                   enable_asserts=True, num_devices=N_CORES,
                   num_swdge_queues=2 if K_RDMA else 1)

    # ---- I/O ----
    mem_d = nc.dram_tensor("mem", [R, D], BF16, kind="ExternalInput")
    bk_d = nc.dram_tensor("bk", [CPC, D], BF16, kind="ExternalInput")
    bv_d = nc.dram_tensor("bv", [CPC, D], BF16, kind="ExternalInput")
    bffn_d = nc.dram_tensor("bffn", [CPC, D], BF16, kind="ExternalInput")
    gbw = CPC if K_RDMA else C
    gbtk_d = nc.dram_tensor("gbtk", [128, NCH * gbw], BF16, kind="ExternalInput")
    img_d = nc.dram_tensor("img", [1, D], F32, kind="ExternalInput")
    imgt_d = nc.dram_tensor("imgt", [128, NCH], F32, kind="ExternalInput")
    ls_d = nc.dram_tensor("ls", [1, 1], F32, kind="ExternalInput")
    ext_d = nc.dram_tensor("ext", [128, NRB * 10], F32, kind="ExternalInput")
    ind_d = nc.dram_tensor("ind", [CPC, NRB * 128], BF16, kind="ExternalInput")
    cmask_d = nc.dram_tensor("cmask", [128, NRB * CPC], BF16, kind="ExternalInput")
    idf_d = nc.dram_tensor("ident_f", [128, 128], F32, kind="ExternalInput")
    idb_d = nc.dram_tensor("ident_b", [128, 128], BF16, kind="ExternalInput")
    peers_d = nc.dram_tensor("peers", [1, 8], I32, kind="ExternalInput")
    probs_d = nc.dram_tensor("probs", [CPC, 1], F32, kind="ExternalOutput")

    waiters = {}
    with tile.TileContext(nc) as tc:
        _body(nc, tc, waiters, mem_d, bk_d, bv_d, bffn_d, gbtk_d, img_d,
              imgt_d, ls_d, ext_d, ind_d, cmask_d, idf_d, idb_d, peers_d,
              probs_d)
    # Remote-data waits are attached AFTER tile scheduling: the single-core
    # scheduling pass would deadlock on semaphores only peers increment.
    for w, (sem, val) in waiters.items():
        w._wait_ge(sem, val)
    nc.compile()
    return nc


def _body(nc, tc, waiters, mem_d, bk_d, bv_d, bffn_d, gbtk_d, img_d, imgt_d,
          ls_d, ext_d, ind_d, cmask_d, idf_d, idb_d, peers_d, probs_d):
    from contextlib import ExitStack
    ctx = ExitStack()
    with ctx:
        cst = ctx.enter_context(tc.tile_pool(name="cst", bufs=1))
        big = ctx.enter_context(tc.tile_pool(name="big", bufs=1))
        mempool = ctx.enter_context(tc.tile_pool(name="mem", bufs=NQB))
        mtpool = ctx.enter_context(tc.tile_pool(name="mt", bufs=8))
        sqpool = ctx.enter_context(tc.tile_pool(name="sq", bufs=2))
        bpool = ctx.enter_context(tc.tile_pool(name="b", bufs=3))
        wpool = ctx.enter_context(tc.tile_pool(name="w", bufs=4))
        small = ctx.enter_context(tc.tile_pool(name="small", bufs=1))
        pst = ctx.enter_context(tc.tile_pool(name="pst", bufs=2, space="PSUM"))
        psd = ctx.enter_context(tc.tile_pool(name="psd", bufs=2, space="PSUM"))
        psa = ctx.enter_context(tc.tile_pool(name="psa", bufs=1, space="PSUM"))
        psx = ctx.enter_context(tc.tile_pool(name="psx", bufs=1, space="PSUM"))

        onesb = nc.const_aps.tensor(1.0, (128, 1), BF16)
        ones1f = nc.const_aps.tensor(1.0, (1, 128), F32)
        onesf_128 = nc.const_aps.tensor(1.0, (128, 1), F32)
        ones1f_cpc = nc.const_aps.tensor(1.0, (1, CPC), F32)

        # ---------- small/critical input DMAs first ----------
        identb = cst.tile([128, 128], BF16)
        nc.sync.dma_start(identb[:], idb_d[:])
        identf = cst.tile([128, 128], F32)
        nc.sync.dma_start(identf[:], idf_d[:])
        gbt = cst.tile([128, NCH, CPC if K_RDMA else C], BF16)
        nc.sync.dma_start(gbt[:], gbtk_d[:])
        imgT = cst.tile([128, NCH], F32)
        nc.sync.dma_start(imgT[:], imgt_d[:])
        img = cst.tile([1, D], F32)
        nc.sync.dma_start(img[:], img_d[:])
        ls = cst.tile([1, 1], F32)
        nc.sync.dma_start(ls[:], ls_d[:])
        peers = cst.tile([1, 8], I32)
        nc.sync.dma_start(peers[:], peers_d[:])
        bkb = cst.tile([CPC, D], BF16)
        nc.sync.dma_start(bkb[:], bk_d[:])
        bvb = cst.tile([CPC, D], BF16)
        nc.sync.dma_start(bvb[:], bv_d[:])
        ext = cst.tile([128, NRB, 10], F32)
        nc.scalar.dma_start(ext[:], ext_d[:])
        ind = cst.tile([CPC, NRB, 128], BF16)
        nc.scalar.dma_start(ind[:], ind_d[:])
        cmask = cst.tile([128, NRB, CPC], BF16)
        nc.scalar.dma_start(cmask[:], cmask_d[:])
        bffnb = cst.tile([CPC, D], BF16)
        nc.scalar.dma_start(bffnb[:], bffn_d[:])

        # ---------- RDMA setup: all descriptor preps early ----------
        # exchange tiles (addresses identical on every core; remote writes
        # land in column d = ring distance from the sender)
        csg = cst.tile([128, 8, NCH], F32)       # colsum partials gather
        colsT = cst.tile([128, NCH], F32)        # local partial colsum
        lg_full = cst.tile([128, 1], F32)        # local logits (pad = -1e30)
        lga = cst.tile([128, 8], F32)            # logits gather
        if K_RDMA:
            rsem_cs = nc.alloc_semaphore("rsem_cs")
            lsem_cs = nc.alloc_semaphore("lsem_cs")
            rsem_lg = nc.alloc_semaphore("rsem_lg")
            lsem_lg = nc.alloc_semaphore("lsem_lg")
            pid_reg = nc.gpsimd.alloc_register("pid_reg")
        else:
            dram = ctx.enter_context(tc.tile_pool(name="dram", bufs=1,
                                                  space="DRAM"))

        # bkvT zero-pad early (Pool work before any Pool RDMA stalls)
        bkvT = cst.tile([128, NCH, 2, CP], BF16)
        nc.gpsimd.memset(bkvT[:], 0.0)

        # ---------- mean(global_bias): partial+exchange or full local ----
        if K_RDMA:
            nc.vector.reduce_sum(colsT[:], gbt[:], axis=AX.X)
            for d in range(1, 8):
                nc.gpsimd.reg_load(pid_reg, peers[:, d:d + 1])
                nc.gpsimd.remote_dma(
                    csg[:, d, :], colsT[:, :], rsem_cs, lsem_cs,
                    pid=pid_reg, routing_id=0, dma_engine_mask=RD_MASK,
                    queue_num=0)
            nc.gpsimd.trigger_dma(count=None, queue_num=0)
            i_csg0 = nc.vector.tensor_copy(csg[:, 0, :], colsT[:])
        else:
            # full local column-sum on DVE (bf16 2x: ~4.2us, off mem path)
            nc.vector.reduce_sum(colsT[:], gbt[:], axis=AX.X)
        if K_RDMA:
            cs_wait = nc.vector.nop(hint="cs_wait", nofuse=True)
            t1 = small.tile([128, 4, NCH], F32)
            i1 = nc.vector.tensor_tensor(t1[:], csg[:, 0:4, :], csg[:, 4:8, :],
                                         op=ALU.add)
            _order(cs_wait, i_csg0)
            waiters[cs_wait] = (rsem_cs, 7)
            _order(i1, cs_wait)
            t2 = small.tile([128, 2, NCH], F32)
            nc.vector.tensor_tensor(t2[:], t1[:, 0:2, :], t1[:, 2:4, :],
                                    op=ALU.add)
            colsum = small.tile([128, NCH], F32)
            nc.vector.tensor_tensor(colsum[:], t2[:, 0, :], t2[:, 1, :],
                                    op=ALU.add)
        else:
            colsum = colsT

        # q in d-major layout: qraw = colsum/C + imgT ; qhat = qraw/||q||
        qrawT = small.tile([128, NCH], F32)
        nc.vector.scalar_tensor_tensor(qrawT[:], colsum[:], 1.0 / C, imgT[:],
                                       op0=ALU.mult, op1=ALU.add)
        qsqp = small.tile([128, 1], F32)
        qn = small.tile([128, NCH], F32)
        nc.scalar.activation(qn[:], qrawT[:], AF.Square, accum_out=qsqp[:])
        qsps = psx.tile([1, 1], F32, tag="x")
        nc.tensor.matmul(qsps[:], onesf_128, qsqp[:], start=True, stop=True)
        qsq = small.tile([1, 1], F32)
        nc.scalar.copy(qsq[:], qsps[:])
        qrs = small.tile([1, 1], F32)
        _emit_rsqrt(nc, small, qrs[:], qsq[:], RSQ_A, 2, "qr")
        qrps = psx.tile([128, 1], F32, tag="x")
        nc.tensor.matmul(qrps[:], ones1f, qrs[:], start=True, stop=True)
        qrsb = small.tile([128, 1], F32)
        nc.scalar.copy(qrsb[:], qrps[:])
        qhatT = cst.tile([128, NCH], BF16)
        nc.vector.tensor_scalar(qhatT[:], qrawT[:], qrsb[:, 0:1], None,
                                op0=ALU.mult)

        # ---------- transposed bias tables bkvT [128, NCH, 2, CP] ----------
        for t, src in ((0, bkb), (1, bvb)):
            for j in range(NCH):
                tp = pst.tile([128, CPC], BF16, tag="tpp")
                nc.tensor.transpose(tp[:], src[:, 128 * j:128 * (j + 1)],
                                    identb[0:CPC, 0:CPC])
                nc.vector.tensor_copy(bkvT[:, j, t, 0:CPC], tp[:])

        # ---------- per-class constants vcat3 = [||bk||^2, ||bv||^2, bk.qhat]
        vcat3 = cst.tile([CPC, 3], BF16)
        nb2f = small.tile([CPC, 2], F32)
        tsq = small.tile([CPC, D], F32, tag="tsq")
        nc.scalar.activation(tsq[:], bkb[:], AF.Square, accum_out=nb2f[:, 0:1])
        tsq2 = small.tile([CPC, D], F32, tag="tsq")
        nc.scalar.activation(tsq2[:], bvb[:], AF.Square, accum_out=nb2f[:, 1:2])
        nc.vector.tensor_copy(vcat3[:, 0:2], nb2f[:])
        bkqps = psx.tile([CPC, 1], F32, tag="x")
        for j in range(NCH):
            nc.tensor.matmul(bkqps[:], bkvT[:, j, 0, 0:CPC], qhatT[:, j:j + 1],
                             start=(j == 0), stop=(j == NCH - 1))
        nc.scalar.copy(vcat3[:, 2:3], bkqps[:])

        # img broadcast to CPC partitions (for the logits dot, done early)
        ibs = cst.tile([CPC, D], F32)
        for h in range(D // 512):
            ibp = psx.tile([CPC, 512], F32, tag="x", name=f"ibp{h}")
            nc.tensor.matmul(ibp[:], ones1f_cpc,
                             img[:, 512 * h:512 * (h + 1)], start=True,
                             stop=True, skip_group_check=True)
            nc.scalar.copy(ibs[:, 512 * h:512 * (h + 1)], ibp[:])

        # ---------- main loop ----------
        nsq_all = big.tile([128, NRB], F32)
        aps = psa.tile([CPC, D], F32)      # A accumulator (2 banks)
        swps = psa.tile([CPC, 1], F32)     # sum of weights
        abf4_tiles = {}
        abf_tiles = {}

        tail_ms = []
        bounds = []
        pos = 0
        for gsz in GROUPS:
            bounds.append((pos, min(NRB, pos + gsz)))
            pos += gsz
            if pos >= NRB:
                break
        for rb_lo, rb_hi in bounds:
            ng = rb_hi - rb_lo
            s_all = wpool.tile([128, GRP, 14], F32, tag="s_all")
            for rb in range(rb_lo, rb_hi):
                r0, nr, c0 = _rb_info(rb)
                qi, qj = divmod(rb, QB)
                if qj == 0:
                    nrb_q = min(QB, NRB - qi * QB)
                    abf4 = mempool.tile([128, QB, D], BF16, tag="abf")
                    full = min(nrb_q, (R - qi * QB * 128) // 128)
                    if full:
                        nc.sync.dma_start(
                            abf4[:, 0:full, :],
                            mem_d.ap()[qi * QB * 128:qi * QB * 128 + full * 128]
                            .rearrange("(i p) d -> p i d", p=128))
                    if full < nrb_q:  # partial last rowblock
                        rr0 = (qi * QB + full) * 128
                        nrr = R - rr0
                        i_tail_ms = nc.gpsimd.memset(abf4[:, full, :], 0.0)
                        tail_ms.append(i_tail_ms)
                        nc.sync.dma_start(abf4[0:nrr, full, :],
                                          mem_d[rr0:rr0 + nrr, :])
                    abf4_tiles[qi] = abf4
                abf = abf4_tiles[qi][:, qj, :]
                abf_tiles[rb] = abf
                # row sums of squares
                sqjunk = sqpool.tile([128, D], BF16, tag="sqjunk")
                sq_eng = SQ_PAT[rb % len(SQ_PAT)]
                if sq_eng == 'd':
                    nc.vector.scalar_tensor_tensor(
                        sqjunk[:], abf[:], 1.0, abf[:],
                        op0=ALU.mult, op1=ALU.mult,
                        accum_out=nsq_all[:, rb:rb + 1])
                elif sq_eng == 'p':
                    nc.gpsimd.scalar_tensor_tensor(
                        sqjunk[:], abf[:], 1.0, abf[:],
                        op0=ALU.mult, op1=ALU.mult,
                        accum_out=nsq_all[:, rb:rb + 1])
                else:
                    nc.scalar.activation(sqjunk[:], abf[:], AF.Square,
                                         accum_out=nsq_all[:, rb:rb + 1])
                # transpose on PE + copyback
                memt = mtpool.tile([128, D], BF16, tag="memt")
                tpp = pst.tile([128, D], BF16, tag="tpp")
                for j in range(NCH):
                    nc.tensor.transpose(tpp[:, 128 * j:128 * (j + 1)],
                                        abf[:, 128 * j:128 * (j + 1)],
                                        identb[:])
                cb_eng = CB_PAT[rb % len(CB_PAT)]
                if cb_eng == 'a':
                    nc.scalar.copy(memt[:], tpp[:])
                elif cb_eng == 'p':
                    nc.gpsimd.tensor_copy(memt[:], tpp[:])
                else:
                    nc.vector.tensor_copy(memt[:], tpp[:])
                # fused dot-pass: [0:10]=bk/bv dots, [10]=q dot, [11:14]=class consts
                dps = psd.tile([128, 14], F32, tag="dps")
                for j in range(NCH):
                    mtj = memt[:, 128 * j:128 * (j + 1)]
                    nc.tensor.matmul(dps[:, 0:10], mtj,
                                     bkvT[:, j, :, c0:c0 + 5],
                                     start=(j == 0), stop=(j == NCH - 1),
                                     skip_group_check=True)
                    nc.tensor.matmul(dps[:, 10:11], mtj, qhatT[:, j:j + 1],
                                     start=(j == 0), stop=(j == NCH - 1),
                                     skip_group_check=True)
                nc.tensor.matmul(dps[:, 11:14], ind[:, rb, :], vcat3[:],
                                 start=True, stop=True, skip_group_check=True)
                if rb % 2 == 0:
                    nc.scalar.copy(s_all[:, rb - rb_lo, :], dps[:])
                else:
                    nc.vector.tensor_copy(s_all[:, rb - rb_lo, :], dps[:])

            # ---- extraction + weights for this group ----
            masked = wpool.tile([128, GRP, 10], F32, tag="masked")
            nc.vector.tensor_tensor(masked[:, 0:ng, :], s_all[:, 0:ng, 0:10],
                                    ext[:, rb_lo:rb_hi, :], op=ALU.mult)
            dotkv = wpool.tile([128, GRP, 2], F32, tag="dotkv")
            nc.vector.reduce_sum(
                dotkv[:, 0:ng, :],
                masked[:, 0:ng, :].rearrange("p g (t f) -> p g t f", t=2),
                axis=AX.X)
            nsq_g = nsq_all[:, rb_lo:rb_hi]
            nkv = wpool.tile([128, GRP, 2], F32, tag="nkv")
            for t in range(2):
                nc.vector.scalar_tensor_tensor(
                    nkv[:, 0:ng, t], dotkv[:, 0:ng, t], 2.0, nsq_g,
                    op0=ALU.mult, op1=ALU.add)
            nc.vector.scalar_tensor_tensor(
                nkv[:, 0:ng, :], nkv[:, 0:ng, :], 1e-12,
                s_all[:, 0:ng, 11:13], op0=ALU.max, op1=ALU.add)
            rkv = wpool.tile([128, GRP, 2], F32, tag="rkv")
            _emit_rsqrt(nc, wpool, rkv[:, 0:ng, :], nkv[:, 0:ng, :], RSQ_A,
                        RSQ_ITERS, "rkv")
            sh = wpool.tile([128, GRP], F32, tag="sh")
            nc.vector.tensor_tensor(sh[:, 0:ng], s_all[:, 0:ng, 10],
                                    s_all[:, 0:ng, 13], op=ALU.add)
            nc.vector.tensor_tensor(sh[:, 0:ng], sh[:, 0:ng],
                                    rkv[:, 0:ng, 0], op=ALU.mult)
            wv = wpool.tile([128, GRP], F32, tag="wv")
            nc.scalar.activation(wv[:, 0:ng], sh[:, 0:ng], AF.Exp, scale=BETA)
            nc.vector.tensor_tensor(wv[:, 0:ng], wv[:, 0:ng], rkv[:, 0:ng, 1],
                                    op=ALU.mult)
            sgn = wpool.tile([128, GRP], F32, tag="sgn")
            nc.scalar.sign(sgn[:, 0:ng], nsq_g)
            # W rowblocks + A accumulation
            for rb in range(rb_lo, rb_hi):
                i = rb - rb_lo
                wrb = bpool.tile([128, CPC], BF16, tag="wrb")
                nc.gpsimd.tensor_scalar(wrb[:], cmask[:, rb, :],
                                        wv[:, i:i + 1], sgn[:, i:i + 1],
                                        op0=ALU.mult, op1=ALU.mult)
                abf = abf_tiles.pop(rb)
                first = rb == 0
                last = rb == NRB - 1
                for h in range(D // 512):
                    nc.tensor.matmul(aps[:, 512 * h:512 * (h + 1)], wrb[:],
                                     abf[:, 512 * h:512 * (h + 1)],
                                     start=first, stop=last,
                                     skip_group_check=True)
                nc.tensor.matmul(swps[:], wrb[:], onesb,
                                 start=first, stop=last,
                                 skip_group_check=True)

        # ---------- tail: a = l2n(l2n(A + SW*bv) + bffn); logits ----------
        sw = small.tile([CPC, 1], F32)
        nc.scalar.copy(sw[:], swps[:])
        apre = small.tile([CPC, D], BF16, tag="apre")
        n1 = small.tile([CPC, 1], F32)
        nc.vector.scalar_tensor_tensor(apre[:], bvb[:], sw[:, 0:1], aps[:],
                                       op0=ALU.mult, op1=ALU.add)
        junk1 = small.tile([CPC, D], BF16, tag="tsqb")
        nc.vector.scalar_tensor_tensor(junk1[:], apre[:], 1.0, apre[:],
                                       op0=ALU.mult, op1=ALU.mult,
                                       accum_out=n1[:])
        r1 = small.tile([CPC, 1], F32)
        _emit_rsqrt_magic(nc, small, r1[:], n1[:], 2, "r1")
        a2 = small.tile([CPC, D], BF16, tag="a2")
        n2 = small.tile([CPC, 1], F32)
        nc.vector.scalar_tensor_tensor(a2[:], apre[:], r1[:, 0:1], bffnb[:],
                                       op0=ALU.mult, op1=ALU.add)
        junk2 = small.tile([CPC, D], BF16, tag="tsqb")
        nc.vector.scalar_tensor_tensor(junk2[:], a2[:], 1.0, a2[:],
                                       op0=ALU.mult, op1=ALU.mult,
                                       accum_out=n2[:])
        r2 = small.tile([CPC, 1], F32)
        _emit_rsqrt(nc, small, r2[:], n2[:], RSQ_B, 2, "r2")
        # dotai = a2 . img  (via pre-broadcast img in psum)
        dotai_h = small.tile([CPC, 2], F32)
        for h in range(D // 512):
            p2 = small.tile([CPC, 512], F32, tag="p2", name=f"p2_{h}")
            nc.vector.scalar_tensor_tensor(
                p2[:], a2[:, 512 * h:512 * (h + 1)], 1.0,
                ibs[:, 512 * h:512 * (h + 1)],
                op0=ALU.mult, op1=ALU.mult, accum_out=dotai_h[:, h:h + 1])
        dotai = small.tile([CPC, 1], F32)
        nc.vector.tensor_tensor(dotai[:], dotai_h[:, 0:1], dotai_h[:, 1:2],
                                op=ALU.add)
        # logits = exp(ls) * r2 * dotai ; pad rows get -1e30
        els = small.tile([1, 1], F32)
        nc.scalar.activation(els[:], ls[:], AF.Exp)
        elsps = psx.tile([CPC, 1], F32, tag="x")
        nc.tensor.matmul(elsps[:], ones1f_cpc, els[:], start=True, stop=True)
        i_pad = nc.vector.memset(lg_full[:], -1e30)
        lgv = small.tile([CPC, 1], F32)
        nc.vector.tensor_tensor(lgv[:], dotai[:], r2[:], op=ALU.mult)
        i_lg = nc.vector.tensor_tensor(lg_full[0:CPC, :], lgv[:], elsps[:],
                                       op=ALU.mult)

        # ---------- logits all-gather + softmax ----------
        if K_RDMA:
            for d in range(1, 8):
                nc.gpsimd.reg_load(pid_reg, peers[:, d:d + 1])
                nc.gpsimd.remote_dma(
                    lga[:, d:d + 1], lg_full[:, :], rsem_lg, lsem_lg,
                    pid=pid_reg, routing_id=0, dma_engine_mask=RD_MASK,
                    queue_num=1)
            nc.gpsimd.trigger_dma(count=None, queue_num=1)
            i_lga0 = nc.vector.tensor_copy(lga[:, 0:1], lg_full[:])
            lg_wait = nc.vector.nop(hint="lg_wait", nofuse=True)
            _order(lg_wait, i_lga0)
            waiters[lg_wait] = (rsem_lg, 7)
        else:
            cc2_in = dram.tile([128, 1], F32)
            cc2_out = dram.tile([128 * N_CORES, 1], F32, addr_space="Shared")
            nc.sync.dma_start(cc2_in[:], lg_full[:])
            nc.gpsimd.collective_compute(
                "AllGather", ALU.bypass,
                replica_groups=[list(range(N_CORES))],
                ins=[cc2_in[:].opt()], outs=[cc2_out[:].opt()],
            )
            nc.sync.dma_start(
                lga[:], cc2_out[:].rearrange("(j p) 1 -> p j", j=N_CORES))
        rmax = small.tile([128, 1], F32)
        i2 = nc.vector.reduce_max(rmax[:], lga[:], axis=AX.X)
        if K_RDMA:
            _order(i2, lg_wait)
        rmps = psx.tile([1, 128], F32, tag="x")
        nc.tensor.transpose(rmps[:], rmax[:], identf[:])
        rms = small.tile([1, 128], F32)
        nc.scalar.copy(rms[:], rmps[:])
        gmax = small.tile([1, 1], F32)
        nc.vector.reduce_max(gmax[:], rms[:], axis=AX.X)
        gmps = psx.tile([128, 1], F32, tag="x")
        nc.tensor.matmul(gmps[:], ones1f, gmax[:], start=True, stop=True)
        negM = small.tile([128, 1], F32)
        nc.scalar.mul(negM[:], gmps[:], -1.0)
        elga = small.tile([128, 8], F32)
        esum = small.tile([128, 1], F32)
        nc.scalar.activation(elga[:], lga[:], AF.Exp, bias=negM[:, 0:1],
                             accum_out=esum[:])
        esps = psx.tile([1, 128], F32, tag="x")
        nc.tensor.transpose(esps[:], esum[:], identf[:])
        ess = small.tile([1, 128], F32)
        nc.scalar.copy(ess[:], esps[:])
        tot = small.tile([1, 1], F32)
        nc.vector.reduce_sum(tot[:], ess[:], axis=AX.X)
        rtot = small.tile([1, 1], F32)
        nc.vector.reciprocal(rtot[:], tot[:])
        rtps = psx.tile([CPC, 1], F32, tag="x")
        nc.tensor.matmul(rtps[:], ones1f_cpc, rtot[:], start=True, stop=True)
        eloc = small.tile([CPC, 1], F32)
        nc.scalar.activation(eloc[:], lg_full[0:CPC, :], AF.Exp,
                             bias=negM[0:CPC, 0:1])
        probs = small.tile([CPC, 1], F32)
        nc.vector.tensor_tensor(probs[:], eloc[:], rtps[:], op=ALU.mult)
        nc.sync.dma_start(probs_d[:], probs[:])


def _order(after, before):
    """Make instruction `after` execute after `before` (same engine)."""
    deps = bass.InstructionNameOrderedSet()
    deps.add(before.ins.name)
    after.ins.add_nosync_dependencies_from(deps)


_NC_CACHE = None


def _get_nc():
    global _NC_CACHE
    if _NC_CACHE is None:
        _NC_CACHE = build_nc()
    return _NC_CACHE


def _make_in_maps(inputs, consts):
    import ml_dtypes
    identb = consts["ident"].astype(ml_dtypes.bfloat16)
    indq = consts["ind"].astype(ml_dtypes.bfloat16)
    cmaskb = consts["cmask"].astype(ml_dtypes.bfloat16)
    memory = np.ascontiguousarray(
        np.asarray(inputs["memory"], np.float32)).astype(ml_dtypes.bfloat16)
    gb = np.asarray(inputs["global_bias"], np.float32)
    img = np.asarray(inputs["img_feat"], np.float32).reshape(1, D)
    imgt = np.ascontiguousarray(img.reshape(NCH, 128).T)
    in_maps = []
    for k in range(N_CORES):
        c0, c1 = k * CPC, (k + 1) * CPC
        # gbtk: d-major transposed gb (own slice for RDMA, full otherwise)
        gsl = gb[c0:c1] if K_RDMA else gb
        gbtk = np.ascontiguousarray(
            gsl.T.reshape(NCH, 128, gsl.shape[0]).transpose(1, 0, 2)
        ).astype(ml_dtypes.bfloat16)
        peers = np.zeros((1, 8), np.int32)
        for d in range(1, 8):
            peers[0, d] = NC_BASE[(k + d) % 8]
        in_maps.append({
            "mem": memory[c0:c1].reshape(R, D),
            "bk": np.ascontiguousarray(inputs["global_bias_key"][c0:c1],
                                       dtype=np.float32).astype(ml_dtypes.bfloat16),
            "bv": np.ascontiguousarray(inputs["global_bias_value"][c0:c1],
                                       dtype=np.float32).astype(ml_dtypes.bfloat16),
            "bffn": np.ascontiguousarray(inputs["global_ffn_bias"][c0:c1],
                                         dtype=np.float32).astype(ml_dtypes.bfloat16),
            "gbtk": gbtk.reshape(128, -1),
            "img": img,
            "imgt": imgt,
            "ls": np.asarray(inputs["logit_scale"], np.float32).reshape(1, 1),
            "ext": consts["ext"],
            "ind": indq,
            "cmask": cmaskb,
            "ident_f": consts["ident"],
            "ident_b": identb,
            "peers": peers,
        })
    return in_maps


def kernel(img_feat, memory, global_bias, global_bias_key, global_bias_value,
           global_ffn_bias, logit_scale, _trace=False):
    nc = _get_nc()
    consts = _host_constants()
    in_maps = _make_in_maps(dict(
        img_feat=img_feat, memory=memory, global_bias=global_bias,
        global_bias_key=global_bias_key, global_bias_value=global_bias_value,
        global_ffn_bias=global_ffn_bias, logit_scale=logit_scale), consts)
    res = run_bass_kernel_spmd(nc, in_maps, core_ids=list(range(N_CORES)),
                               trace=_trace)
    out = np.concatenate([res.results[k]["probs"][:, 0] for k in range(N_CORES)])
    kernel._last_result = res
    return out.reshape(1, C).astype(np.float32)


# revision 6
# speedup vs baseline: 1.0274x; 1.0274x over previous
"""Trainium2 Bass kernel for DualMem retrieval (exp-cosine kNN memory head), v2.

Contract: kernel(**inputs) takes FULL numpy inputs, returns FULL [1, C]
softmax output.  Class axis sharded over 8 NeuronCores.

Math (reference):
  q   = l2norm(img + mean_c(global_bias))                       [1, D]
  K/V = l2norm(mem + bias_{k,v}[c]) masked where row(mem)==0    [C,Mt,D]
  sim = exp(-beta*(1 - q.K))                                    [C,Mt]
  a   = l2norm(l2norm(sim @ V) + ffn_bias)                      [C, D]
  out = softmax(exp(ls) * a @ img)                              [1, C]

Structural changes vs the first-generation kernel (110.7us -> ~99us):
  - mean(global_bias) from a host-transposed d-major copy of gb reduced
    locally on DVE (bf16, 2x mode) -- replaces a 12.6us f32 HWDGE load +
    ~14us of f32 colsum matmuls on the PE; qhat is ready by ~11us.
  - mem/bias tables stored bf16 in DRAM (host cast): halves DMA bytes and
    allows the batches to stream on the SP HWDGE queue (casting DMAs are
    SWDGE-only), keeping GPSIMD free; all 33 rowblocks are buffered in
    SBUF so the DMA device never stalls on consumers.
  - fused dot-pass: bk/bv dots, q dot and the per-class constant gather
    ([||bk||^2, ||bv||^2, bk.qhat] via one fp8 one-hot matmul) accumulate
    into one [128,14] psum -> one evacuation copy per rowblock.
  - extraction does k/v lanes together on [128,g,2] tiles; rsqrt is a
    quadratic seed + 1 Newton step (tolerance is 2e-2); weight rowblocks
    are built on GPSIMD.
  - engine balance: transpose copybacks on DVE, row-square reductions
    rotated over ACT/GPSIMD (K_SQ/K_CB env knobs).
  - K_RDMA=1 (sim-only experiment; the GPSIMD remote-DMA ucode is not
    available in this runner's NEFF image) replaces both AllGather
    collectives (15us constant overhead each) with direct SBUF remote
    DMAs between the 8 cores: ~80us in the cost model.
"""

import os
import sys

sys.path.insert(0, "/opt/trn_rl_repo")

import numpy as np

import concourse.bass as bass
import concourse.mybir as mybir
import concourse.tile as tile
from concourse import bacc
from concourse.bass_utils import run_bass_kernel_spmd

F32 = mybir.dt.float32
BF16 = mybir.dt.bfloat16
FP8 = mybir.dt.float8e4
I32 = mybir.dt.int32
AF = mybir.ActivationFunctionType
ALU = mybir.AluOpType
AX = mybir.AxisListType

BETA = 5.5
N_CORES = 8
C, MT, D = 1000, 33, 1024
CPC = C // N_CORES          # classes per core
R = CPC * MT                # rows per core
NRB = (R + 127) // 128      # rowblocks per core
NCH = D // 128              # 128-wide d-chunks
QB = 4                      # rowblocks per mem DMA batch
NQB = (NRB + QB - 1) // QB
CP = CPC + 7                # padded class dim so [c0:c0+5] slices stay in range

GROUPS = tuple(int(x) for x in os.environ.get('K_GROUPS', '11,11,11').split(','))
GRP = max(GROUPS)
# per-rowblock square engine: 'd'=DVE  'a'=ACT   (pattern repeats)
SQ_PAT = os.environ.get('K_SQ', 'adaada')
# per-rowblock transpose-copyback engine: 'd'=DVE 'a'=ACT
CB_PAT = os.environ.get('K_CB', 'dddddd')
K_RDMA = int(os.environ.get('K_RDMA', '0'))
RSQ_ITERS = int(os.environ.get('K_RSQI', '1'))

# logical core -> physical nc index (TRN2 period-8 map)
NC_BASE = (0, 1, 2, 3, 6, 7, 4, 5)
RD_MASK = 0x10   # DMA engine 4: reaches both same-die and cross-die peers

RSQ_A = (0.05888337527349581, -3.735601567857182e-05, 1.02184149458168e-08)
RSQ_B = (1.6460793992359617, -0.7401760506078425, 0.1316746462210596)
MAGIC = 0x5f3759df


def _rb_info(rb):
    r0 = rb * 128
    nr = min(128, R - r0)
    c0 = r0 // MT
    return r0, nr, c0


def _host_constants():
    """Compile-time one-hot tensors from the class-major row layout
    (row r of a core = class r//MT, slot r%MT)."""
    ext = np.zeros((128, NRB, 10), np.float32)
    ind = np.zeros((CPC, NRB, 128), np.float32)
    cmask = np.zeros((128, NRB, CPC), np.float32)
    for rb in range(NRB):
        r0, nr, c0 = _rb_info(rb)
        for p in range(nr):
            c = (r0 + p) // MT
            jc = c - c0
            ext[p, rb, jc] = 1.0
            ext[p, rb, 5 + jc] = 1.0
            ind[c, rb, p] = 1.0
            cmask[p, rb, c] = 1.0
    ident = np.eye(128, dtype=np.float32)
    return {
        "ext": ext.reshape(128, NRB * 10),
        "ind": ind.reshape(CPC, NRB * 128),
        "cmask": cmask.reshape(128, NRB * CPC),
        "ident": ident,
    }


def _emit_rsqrt(nc, pool, out, x, coef, iters, tag):
    """out = x**-0.5 via quadratic seed (fitted range) + Newton steps."""
    c0, c1, c2 = coef
    shp, dt = list(x.shape), F32
    t = pool.tile(shp, dt, tag=tag + "t")
    nc.vector.tensor_scalar(t[:], x, c2, c1, op0=ALU.mult, op1=ALU.add)
    y = pool.tile(shp, dt, tag=tag + "y")
    nc.vector.scalar_tensor_tensor(y[:], t[:], 1.0, x, op0=ALU.mult, op1=ALU.mult)
    nc.vector.tensor_scalar(y[:], y[:], c0, None, op0=ALU.add)
    for _ in range(iters):
        a = pool.tile(shp, dt, tag=tag + "a")
        nc.vector.scalar_tensor_tensor(a[:], y[:], 1.0, y[:], op0=ALU.mult,
                                       op1=ALU.mult)
        nc.vector.scalar_tensor_tensor(a[:], a[:], -0.5, x, op0=ALU.mult,
                                       op1=ALU.mult)
        nc.vector.tensor_scalar(a[:], a[:], 1.5, None, op0=ALU.add)
        nc.vector.tensor_tensor(y[:], y[:], a[:], op=ALU.mult)
    nc.vector.tensor_copy(out, y[:])


def _emit_rsqrt_magic(nc, pool, out, x, iters, tag):
    """out = x**-0.5 via int bit-magic seed + Newton (any positive range)."""
    shp = list(x.shape)
    yi = pool.tile(shp, mybir.dt.int32, tag=tag + "i")
    nc.vector.tensor_scalar(yi[:], x.bitcast(mybir.dt.int32), 1, None,
                            op0=ALU.logical_shift_right)
    nc.vector.tensor_scalar(yi[:], yi[:], MAGIC, -1, op0=ALU.subtract,
                            op1=ALU.mult)
    y = yi[:].bitcast(F32)
    for _ in range(iters):
        a = pool.tile(shp, F32, tag=tag + "a")
        nc.vector.scalar_tensor_tensor(a[:], y, 1.0, y, op0=ALU.mult,
                                       op1=ALU.mult)
        nc.vector.scalar_tensor_tensor(a[:], a[:], -0.5, x, op0=ALU.mult,
                                       op1=ALU.mult)
        nc.vector.tensor_scalar(a[:], a[:], 1.5, None, op0=ALU.add)
        nc.vector.tensor_tensor(y, y, a[:], op=ALU.mult)
    nc.vector.tensor_copy(out, y)


def build_nc():
    nc = bacc.Bacc("TRN2", target_bir_lowering=False, debug=False,
                   enable_asserts=True, num_devices=N_CORES,
                   num_swdge_queues=2 if K_RDMA else 1)

    # ---- I/O ----
    mem_d = nc.dram_tensor("mem", [R, D], BF16, kind="ExternalInput")
    bk_d = nc.dram_tensor("bk", [CPC, D], BF16, kind="ExternalInput")
    bv_d = nc.dram_tensor("bv", [CPC, D], BF16, kind="ExternalInput")
    bffn_d = nc.dram_tensor("bffn", [CPC, D], BF16, kind="ExternalInput")
    gbw = CPC if K_RDMA else C
    gbtk_d = nc.dram_tensor("gbtk", [128, NCH * gbw], BF16, kind="ExternalInput")
    img_d = nc.dram_tensor("img", [1, D], F32, kind="ExternalInput")
    imgt_d = nc.dram_tensor("imgt", [128, NCH], F32, kind="ExternalInput")
    ls_d = nc.dram_tensor("ls", [1, 1], F32, kind="ExternalInput")
    ext_d = nc.dram_tensor("ext", [128, NRB * 10], F32, kind="ExternalInput")
    ind_d = nc.dram_tensor("ind", [CPC, NRB * 128], BF16, kind="ExternalInput")
    cmask_d = nc.dram_tensor("cmask", [128, NRB * CPC], BF16, kind="ExternalInput")
    idf_d = nc.dram_tensor("ident_f", [128, 128], F32, kind="ExternalInput")
    idb_d = nc.dram_tensor("ident_b", [128, 128], BF16, kind="ExternalInput")
    peers_d = nc.dram_tensor("peers", [1, 8], I32, kind="ExternalInput")
    probs_d = nc.dram_tensor("probs", [CPC, 1], F32, kind="ExternalOutput")

    waiters = {}
    with tile.TileContext(nc) as tc:
        _body(nc, tc, waiters, mem_d, bk_d, bv_d, bffn_d, gbtk_d, img_d,
              imgt_d, ls_d, ext_d, ind_d, cmask_d, idf_d, idb_d, peers_d,
              probs_d)
    # Remote-data waits are attached AFTER tile scheduling: the single-core
    # scheduling pass would deadlock on semaphores only peers increment.
    for w, (sem, val) in waiters.items():
        w._wait_ge(sem, val)
    nc.compile()
    return nc


def _body(nc, tc, waiters, mem_d, bk_d, bv_d, bffn_d, gbtk_d, img_d, imgt_d,
          ls_d, ext_d, ind_d, cmask_d, idf_d, idb_d, peers_d, probs_d):
    from contextlib import ExitStack
    ctx = ExitStack()
    with ctx:
        cst = ctx.enter_context(tc.tile_pool(name="cst", bufs=1))
        big = ctx.enter_context(tc.tile_pool(name="big", bufs=1))
        mempool = ctx.enter_context(tc.tile_pool(name="mem", bufs=NQB))
        mtpool = ctx.enter_context(tc.tile_pool(name="mt", bufs=8))
        sqpool = ctx.enter_context(tc.tile_pool(name="sq", bufs=2))
        bpool = ctx.enter_context(tc.tile_pool(name="b", bufs=3))
        wpool = ctx.enter_context(tc.tile_pool(name="w", bufs=4))
        small = ctx.enter_context(tc.tile_pool(name="small", bufs=1))
        pst = ctx.enter_context(tc.tile_pool(name="pst", bufs=2, space="PSUM"))
        psd = ctx.enter_context(tc.tile_pool(name="psd", bufs=2, space="PSUM"))
        psa = ctx.enter_context(tc.tile_pool(name="psa", bufs=1, space="PSUM"))
        psx = ctx.enter_context(tc.tile_pool(name="psx", bufs=1, space="PSUM"))

        onesb = nc.const_aps.tensor(1.0, (128, 1), BF16)
        ones1f = nc.const_aps.tensor(1.0, (1, 128), F32)
        onesf_128 = nc.const_aps.tensor(1.0, (128, 1), F32)
        ones1f_cpc = nc.const_aps.tensor(1.0, (1, CPC), F32)

        # ---------- small/critical input DMAs first ----------
        identb = cst.tile([128, 128], BF16)
        nc.sync.dma_start(identb[:], idb_d[:])
        identf = cst.tile([128, 128], F32)
        nc.sync.dma_start(identf[:], idf_d[:])
        gbt = cst.tile([128, NCH, CPC if K_RDMA else C], BF16)
        nc.sync.dma_start(gbt[:], gbtk_d[:])
        imgT = cst.tile([128, NCH], F32)
        nc.sync.dma_start(imgT[:], imgt_d[:])
        img = cst.tile([1, D], F32)
        nc.sync.dma_start(img[:], img_d[:])
        ls = cst.tile([1, 1], F32)
        nc.sync.dma_start(ls[:], ls_d[:])
        peers = cst.tile([1, 8], I32)
        nc.sync.dma_start(peers[:], peers_d[:])
        bkb = cst.tile([CPC, D], BF16)
        nc.sync.dma_start(bkb[:], bk_d[:])
        bvb = cst.tile([CPC, D], BF16)
        nc.sync.dma_start(bvb[:], bv_d[:])
        ext = cst.tile([128, NRB, 10], F32)
        nc.scalar.dma_start(ext[:], ext_d[:])
        ind = cst.tile([CPC, NRB, 128], BF16)
        nc.scalar.dma_start(ind[:], ind_d[:])
        cmask = cst.tile([128, NRB, CPC], BF16)
        nc.scalar.dma_start(cmask[:], cmask_d[:])
        bffnb = cst.tile([CPC, D], BF16)
        nc.scalar.dma_start(bffnb[:], bffn_d[:])

        # ---------- RDMA setup: all descriptor preps early ----------
        # exchange tiles (addresses identical on every core; remote writes
        # land in column d = ring distance from the sender)
        csg = cst.tile([128, 8, NCH], F32)       # colsum partials gather
        colsT = cst.tile([128, NCH], F32)        # local partial colsum
        lg_full = cst.tile([128, 1], F32)        # local logits (pad = -1e30)
        lga = cst.tile([128, 8], F32)            # logits gather
        if K_RDMA:
            rsem_cs = nc.alloc_semaphore("rsem_cs")
            lsem_cs = nc.alloc_semaphore("lsem_cs")
            rsem_lg = nc.alloc_semaphore("rsem_lg")
            lsem_lg = nc.alloc_semaphore("lsem_lg")
            pid_reg = nc.gpsimd.alloc_register("pid_reg")
        else:
            dram = ctx.enter_context(tc.tile_pool(name="dram", bufs=1,
                                                  space="DRAM"))

        # bkvT zero-pad early (Pool work before any Pool RDMA stalls)
        bkvT = cst.tile([128, NCH, 2, CP], BF16)
        nc.gpsimd.memset(bkvT[:], 0.0)

        # ---------- mean(global_bias): partial+exchange or full local ----
        if K_RDMA:
            nc.vector.reduce_sum(colsT[:], gbt[:], axis=AX.X)
            for d in range(1, 8):
                nc.gpsimd.reg_load(pid_reg, peers[:, d:d + 1])
                nc.gpsimd.remote_dma(
                    csg[:, d, :], colsT[:, :], rsem_cs, lsem_cs,
                    pid=pid_reg, routing_id=0, dma_engine_mask=RD_MASK,
                    queue_num=0)
            nc.gpsimd.trigger_dma(count=None, queue_num=0)
            i_csg0 = nc.vector.tensor_copy(csg[:, 0, :], colsT[:])
        else:
            # full local column-sum on DVE (bf16 2x: ~4.2us, off mem path)
            nc.vector.reduce_sum(colsT[:], gbt[:], axis=AX.X)
        if K_RDMA:
            cs_wait = nc.vector.nop(hint="cs_wait", nofuse=True)
            t1 = small.tile([128, 4, NCH], F32)
            i1 = nc.vector.tensor_tensor(t1[:], csg[:, 0:4, :], csg[:, 4:8, :],
                                         op=ALU.add)
            _order(cs_wait, i_csg0)
            waiters[cs_wait] = (rsem_cs, 7)
            _order(i1, cs_wait)
            t2 = small.tile([128, 2, NCH], F32)
            nc.vector.tensor_tensor(t2[:], t1[:, 0:2, :], t1[:, 2:4, :],
                                    op=ALU.add)
            colsum = small.tile([128, NCH], F32)
            nc.vector.tensor_tensor(colsum[:], t2[:, 0, :], t2[:, 1, :],
                                    op=ALU.add)
        else:
            colsum = colsT

        # q in d-major layout: qraw = colsum/C + imgT ; qhat = qraw/||q||
        qrawT = small.tile([128, NCH], F32)
        nc.vector.scalar_tensor_tensor(qrawT[:], colsum[:], 1.0 / C, imgT[:],
                                       op0=ALU.mult, op1=ALU.add)
        qsqp = small.tile([128, 1], F32)
        qn = small.tile([128, NCH], F32)
        nc.scalar.activation(qn[:], qrawT[:], AF.Square, accum_out=qsqp[:])
        qsps = psx.tile([1, 1], F32, tag="x")
        nc.tensor.matmul(qsps[:], onesf_128, qsqp[:], start=True, stop=True)
        qsq = small.tile([1, 1], F32)
        nc.scalar.copy(qsq[:], qsps[:])
        qrs = small.tile([1, 1], F32)
        _emit_rsqrt(nc, small, qrs[:], qsq[:], RSQ_A, 2, "qr")
        qrps = psx.tile([128, 1], F32, tag="x")
        nc.tensor.matmul(qrps[:], ones1f, qrs[:], start=True, stop=True)
        qrsb = small.tile([128, 1], F32)
        nc.scalar.copy(qrsb[:], qrps[:])
        qhatT = cst.tile([128, NCH], BF16)
        nc.vector.tensor_scalar(qhatT[:], qrawT[:], qrsb[:, 0:1], None,
                                op0=ALU.mult)

        # ---------- transposed bias tables bkvT [128, NCH, 2, CP] ----------
        for t, src in ((0, bkb), (1, bvb)):
            for j in range(NCH):
                tp = pst.tile([128, CPC], BF16, tag="tpp")
                nc.tensor.transpose(tp[:], src[:, 128 * j:128 * (j + 1)],
                                    identb[0:CPC, 0:CPC])
                nc.vector.tensor_copy(bkvT[:, j, t, 0:CPC], tp[:])

        # ---------- per-class constants vcat3 = [||bk||^2, ||bv||^2, bk.qhat]
        vcat3 = cst.tile([CPC, 3], BF16)
        nb2f = small.tile([CPC, 2], F32)
        tsq = small.tile([CPC, D], F32, tag="tsq")
        nc.scalar.activation(tsq[:], bkb[:], AF.Square, accum_out=nb2f[:, 0:1])
        tsq2 = small.tile([CPC, D], F32, tag="tsq")
        nc.scalar.activation(tsq2[:], bvb[:], AF.Square, accum_out=nb2f[:, 1:2])
        nc.vector.tensor_copy(vcat3[:, 0:2], nb2f[:])
        bkqps = psx.tile([CPC, 1], F32, tag="x")
        for j in range(NCH):
            nc.tensor.matmul(bkqps[:], bkvT[:, j, 0, 0:CPC], qhatT[:, j:j + 1],
                             start=(j == 0), stop=(j == NCH - 1))
        nc.scalar.copy(vcat3[:, 2:3], bkqps[:])

        # img broadcast to CPC partitions (for the logits dot, done early)
        ibs = cst.tile([CPC, D], F32)
        for h in range(D // 512):
            ibp = psx.tile([CPC, 512], F32, tag="x", name=f"ibp{h}")
            nc.tensor.matmul(ibp[:], ones1f_cpc,
                             img[:, 512 * h:512 * (h + 1)], start=True,
                             stop=True, skip_group_check=True)
            nc.scalar.copy(ibs[:, 512 * h:512 * (h + 1)], ibp[:])

        # ---------- main loop ----------
        nsq_all = big.tile([128, NRB], F32)
        aps = psa.tile([CPC, D], F32)      # A accumulator (2 banks)
        swps = psa.tile([CPC, 1], F32)     # sum of weights
        abf4_tiles = {}
        abf_tiles = {}

        tail_ms = []
        bounds = []
        pos = 0
        for gsz in GROUPS:
            bounds.append((pos, min(NRB, pos + gsz)))
            pos += gsz
            if pos >= NRB:
                break
        for rb_lo, rb_hi in bounds:
            ng = rb_hi - rb_lo
            s_all = wpool.tile([128, GRP, 14], F32, tag="s_all")
            for rb in range(rb_lo, rb_hi):
                r0, nr, c0 = _rb_info(rb)
                qi, qj = divmod(rb, QB)
                if qj == 0:
                    nrb_q = min(QB, NRB - qi * QB)
                    abf4 = mempool.tile([128, QB, D], BF16, tag="abf")
                    full = min(nrb_q, (R - qi * QB * 128) // 128)
                    if full:
                        nc.sync.dma_start(
                            abf4[:, 0:full, :],
                            mem_d.ap()[qi * QB * 128:qi * QB * 128 + full * 128]
                            .rearrange("(i p) d -> p i d", p=128))
                    if full < nrb_q:  # partial last rowblock
                        rr0 = (qi * QB + full) * 128
                        nrr = R - rr0
                        i_tail_ms = nc.gpsimd.memset(abf4[:, full, :], 0.0)
                        tail_ms.append(i_tail_ms)
                        nc.sync.dma_start(abf4[0:nrr, full, :],
                                          mem_d[rr0:rr0 + nrr, :])
                    abf4_tiles[qi] = abf4
                abf = abf4_tiles[qi][:, qj, :]
                abf_tiles[rb] = abf
                # row sums of squares
                sqjunk = sqpool.tile([128, D], BF16, tag="sqjunk")
                sq_eng = SQ_PAT[rb % len(SQ_PAT)]
                if sq_eng == 'd':
                    nc.vector.scalar_tensor_tensor(
                        sqjunk[:], abf[:], 1.0, abf[:],
                        op0=ALU.mult, op1=ALU.mult,
                        accum_out=nsq_all[:, rb:rb + 1])
                elif sq_eng == 'p':
                    nc.gpsimd.scalar_tensor_tensor(
                        sqjunk[:], abf[:], 1.0, abf[:],
                        op0=ALU.mult, op1=ALU.mult,
                        accum_out=nsq_all[:, rb:rb + 1])
                else:
                    nc.scalar.activation(sqjunk[:], abf[:], AF.Square,
                                         accum_out=nsq_all[:, rb:rb + 1])
                # transpose on PE + copyback
                memt = mtpool.tile([128, D], BF16, tag="memt")
                tpp = pst.tile([128, D], BF16, tag="tpp")
                for j in range(NCH):
                    nc.tensor.transpose(tpp[:, 128 * j:128 * (j + 1)],
                                        abf[:, 128 * j:128 * (j + 1)],
                                        identb[:])
                cb_eng = CB_PAT[rb % len(CB_PAT)]
                if cb_eng == 'a':
                    nc.scalar.copy(memt[:], tpp[:])
                elif cb_eng == 'p':
                    nc.gpsimd.tensor_copy(memt[:], tpp[:])
                else:
                    nc.vector.tensor_copy(memt[:], tpp[:])
                # fused dot-pass: [0:10]=bk/bv dots, [10]=q dot, [11:14]=class consts
                dps = psd.tile([128, 14], F32, tag="dps")
                for j in range(NCH):
                    mtj = memt[:, 128 * j:128 * (j + 1)]
                    nc.tensor.matmul(dps[:, 0:10], mtj,
                                     bkvT[:, j, :, c0:c0 + 5],
                                     start=(j == 0), stop=(j == NCH - 1),
                                     skip_group_check=True)
                    nc.tensor.matmul(dps[:, 10:11], mtj, qhatT[:, j:j + 1],
                                     start=(j == 0), stop=(j == NCH - 1),
                                     skip_group_check=True)
                nc.tensor.matmul(dps[:, 11:14], ind[:, rb, :], vcat3[:],
                                 start=True, stop=True, skip_group_check=True)
                if rb % 2 == 0:
                    nc.scalar.copy(s_all[:, rb - rb_lo, :], dps[:])
                else:
                    nc.vector.tensor_copy(s_all[:, rb - rb_lo, :], dps[:])

            # ---- extraction + weights for this group ----
            masked = wpool.tile([128, GRP, 10], F32, tag="masked")
            nc.vector.tensor_tensor(masked[:, 0:ng, :], s_all[:, 0:ng, 0:10],
                                    ext[:, rb_lo:rb_hi, :], op=ALU.mult)
            dotkv = wpool.tile([128, GRP, 2], F32, tag="dotkv")
            nc.vector.reduce_sum(
                dotkv[:, 0:ng, :],
                masked[:, 0:ng, :].rearrange("p g (t f) -> p g t f", t=2),
                axis=AX.X)
            nsq_g = nsq_all[:, rb_lo:rb_hi]
            nkv = wpool.tile([128, GRP, 2], F32, tag="nkv")
            for t in range(2):
                nc.vector.scalar_tensor_tensor(
                    nkv[:, 0:ng, t], dotkv[:, 0:ng, t], 2.0, nsq_g,
                    op0=ALU.mult, op1=ALU.add)
            nc.vector.scalar_tensor_tensor(
                nkv[:, 0:ng, :], nkv[:, 0:ng, :], 1e-12,
                s_all[:, 0:ng, 11:13], op0=ALU.max, op1=ALU.add)
            rkv = wpool.tile([128, GRP, 2], F32, tag="rkv")
            _emit_rsqrt(nc, wpool, rkv[:, 0:ng, :], nkv[:, 0:ng, :], RSQ_A,
                        RSQ_ITERS, "rkv")
            sh = wpool.tile([128, GRP], F32, tag="sh")
            nc.vector.tensor_tensor(sh[:, 0:ng], s_all[:, 0:ng, 10],
                                    s_all[:, 0:ng, 13], op=ALU.add)
            nc.vector.tensor_tensor(sh[:, 0:ng], sh[:, 0:ng],
                                    rkv[:, 0:ng, 0], op=ALU.mult)
            wv = wpool.tile([128, GRP], F32, tag="wv")
            nc.scalar.activation(wv[:, 0:ng], sh[:, 0:ng], AF.Exp, scale=BETA)
            nc.vector.tensor_tensor(wv[:, 0:ng], wv[:, 0:ng], rkv[:, 0:ng, 1],
                                    op=ALU.mult)
            sgn = wpool.tile([128, GRP], F32, tag="sgn")
            nc.scalar.sign(sgn[:, 0:ng], nsq_g)
            # W rowblocks + A accumulation
            for rb in range(rb_lo, rb_hi):
                i = rb - rb_lo
                wrb = bpool.tile([128, CPC], BF16, tag="wrb")
                nc.vector.tensor_scalar(wrb[:], cmask[:, rb, :],
                                        wv[:, i:i + 1], sgn[:, i:i + 1],
                                        op0=ALU.mult, op1=ALU.mult)
                abf = abf_tiles.pop(rb)
                first = rb == 0
                last = rb == NRB - 1
                for h in range(D // 512):
                    nc.tensor.matmul(aps[:, 512 * h:512 * (h + 1)], wrb[:],
                                     abf[:, 512 * h:512 * (h + 1)],
                                     start=first, stop=last,
                                     skip_group_check=True)
                nc.tensor.matmul(swps[:], wrb[:], onesb,
                                 start=first, stop=last,
                                 skip_group_check=True)

        # ---------- tail: a = l2n(l2n(A + SW*bv) + bffn); logits ----------
        sw = small.tile([CPC, 1], F32)
        nc.scalar.copy(sw[:], swps[:])
        apre = small.tile([CPC, D], BF16, tag="apre")
        n1 = small.tile([CPC, 1], F32)
        nc.vector.scalar_tensor_tensor(apre[:], bvb[:], sw[:, 0:1], aps[:],
                                       op0=ALU.mult, op1=ALU.add)
        junk1 = small.tile([CPC, D], BF16, tag="tsqb")
        nc.vector.scalar_tensor_tensor(junk1[:], apre[:], 1.0, apre[:],
                                       op0=ALU.mult, op1=ALU.mult,
                                       accum_out=n1[:])
        r1 = small.tile([CPC, 1], F32)
        _emit_rsqrt_magic(nc, small, r1[:], n1[:], 2, "r1")
        a2 = small.tile([CPC, D], BF16, tag="a2")
        n2 = small.tile([CPC, 1], F32)
        nc.vector.scalar_tensor_tensor(a2[:], apre[:], r1[:, 0:1], bffnb[:],
                                       op0=ALU.mult, op1=ALU.add)
        junk2 = small.tile([CPC, D], BF16, tag="tsqb")
        nc.vector.scalar_tensor_tensor(junk2[:], a2[:], 1.0, a2[:],
                                       op0=ALU.mult, op1=ALU.mult,
                                       accum_out=n2[:])
        r2 = small.tile([CPC, 1], F32)
        _emit_rsqrt(nc, small, r2[:], n2[:], RSQ_B, 2, "r2")
        # dotai = a2 . img  (via pre-broadcast img in psum)
        dotai_h = small.tile([CPC, 2], F32)
        for h in range(D // 512):
            p2 = small.tile([CPC, 512], F32, tag="p2", name=f"p2_{h}")
            nc.vector.scalar_tensor_tensor(
                p2[:], a2[:, 512 * h:512 * (h + 1)], 1.0,
                ibs[:, 512 * h:512 * (h + 1)],
                op0=ALU.mult, op1=ALU.mult, accum_out=dotai_h[:, h:h + 1])
        dotai = small.tile([CPC, 1], F32)
        nc.vector.tensor_tensor(dotai[:], dotai_h[:, 0:1], dotai_h[:, 1:2],
                                op=ALU.add)
        # logits = exp(ls) * r2 * dotai ; pad rows get -1e30
        els = small.tile([1, 1], F32)
        nc.scalar.activation(els[:], ls[:], AF.Exp)
        elsps = psx.tile([CPC, 1], F32, tag="x")
        nc.tensor.matmul(elsps[:], ones1f_cpc, els[:], start=True, stop=True)
        i_pad = nc.vector.memset(lg_full[:], -1e30)
        lgv = small.tile([CPC, 1], F32)
        nc.vector.tensor_tensor(lgv[:], dotai[:], r2[:], op=ALU.mult)
        i_lg = nc.vector.tensor_tensor(lg_full[0:CPC, :], lgv[:], elsps[:],
                                       op=ALU.mult)

        # ---------- logits all-gather + softmax ----------
        if K_RDMA:
            for d in range(1, 8):
                nc.gpsimd.reg_load(pid_reg, peers[:, d:d + 1])
                nc.gpsimd.remote_dma(
                    lga[:, d:d + 1], lg_full[:, :], rsem_lg, lsem_lg,
                    pid=pid_reg, routing_id=0, dma_engine_mask=RD_MASK,
                    queue_num=1)
            nc.gpsimd.trigger_dma(count=None, queue_num=1)
            i_lga0 = nc.vector.tensor_copy(lga[:, 0:1], lg_full[:])
            lg_wait = nc.vector.nop(hint="lg_wait", nofuse=True)
            _order(lg_wait, i_lga0)
            waiters[lg_wait] = (rsem_lg, 7)
        else:
            cc2_in = dram.tile([128, 1], F32)
            cc2_out = dram.tile([128 * N_CORES, 1], F32, addr_space="Shared")
            nc.sync.dma_start(cc2_in[:], lg_full[:])
            nc.gpsimd.collective_compute(
                "AllGather", ALU.bypass,
                replica_groups=[list(range(N_CORES))],
                ins=[cc2_in[:].opt()], outs=[cc2_out[:].opt()],
            )
            nc.sync.dma_start(
                lga[:], cc2_out[:].rearrange("(j p) 1 -> p j", j=N_CORES))
        rmax = small.tile([128, 1], F32)
        i2 = nc.vector.reduce_max(rmax[:], lga[:], axis=AX.X)
        if K_RDMA:
            _order(i2, lg_wait)
        rmps = psx.tile([1, 128], F32, tag="x")
        nc.tensor.transpose(rmps[:], rmax[:], identf[:])
        rms = small.tile([1, 128], F32)
        nc.scalar.copy(rms[:], rmps[:])
        gmax = small.tile([1, 1], F32)
        nc.vector.reduce_max(gmax[:], rms[:], axis=AX.X)
        gmps = psx.tile([128, 1], F32, tag="x")
        nc.tensor.matmul(gmps[:], ones1f, gmax[:], start=True, stop=True)
        negM = small.tile([128, 1], F32)
        nc.scalar.mul(negM[:], gmps[:], -1.0)
        elga = small.tile([128, 8], F32)
        esum = small.tile([128, 1], F32)
        nc.scalar.activation(elga[:], lga[:], AF.Exp, bias=negM[:, 0:1],
                             accum_out=esum[:])
        esps = psx.tile([1, 128], F32, tag="x")
        nc.tensor.transpose(esps[:], esum[:], identf[:])
        ess = small.tile([1, 128], F32)
        nc.scalar.copy(ess[:], esps[:])
        tot = small.tile([1, 1], F32)
        nc.vector.reduce_sum(tot[:], ess[:], axis=AX.X)
        rtot = small.tile([1, 1], F32)
        nc.vector.reciprocal(rtot[:], tot[:])
        rtps = psx.tile([CPC, 1], F32, tag="x")
        nc.tensor.matmul(rtps[:], ones1f_cpc, rtot[:], start=True, stop=True)
        eloc = small.tile([CPC, 1], F32)
        nc.scalar.activation(eloc[:], lg_full[0:CPC, :], AF.Exp,
                             bias=negM[0:CPC, 0:1])
        probs = small.tile([CPC, 1], F32)
        nc.vector.tensor_tensor(probs[:], eloc[:], rtps[:], op=ALU.mult)
        nc.sync.dma_start(probs_d[:], probs[:])


def _order(after, before):
    """Make instruction `after` execute after `before` (same engine)."""
    deps = bass.InstructionNameOrderedSet()
    deps.add(before.ins.name)
    after.ins.add_nosync_dependencies_from(deps)


_NC_CACHE = None


def _get_nc():
    global _NC_CACHE
    if _NC_CACHE is None:
        _NC_CACHE = build_nc()
    return _NC_CACHE


def _make_in_maps(inputs, consts):
    import ml_dtypes
    identb = consts["ident"].astype(ml_dtypes.bfloat16)
    indq = consts["ind"].astype(ml_dtypes.bfloat16)
    cmaskb = consts["cmask"].astype(ml_dtypes.bfloat16)
    memory = np.ascontiguousarray(
        np.asarray(inputs["memory"], np.float32)).astype(ml_dtypes.bfloat16)
    gb = np.asarray(inputs["global_bias"], np.float32)
    img = np.asarray(inputs["img_feat"], np.float32).reshape(1, D)
    imgt = np.ascontiguousarray(img.reshape(NCH, 128).T)
    in_maps = []
    for k in range(N_CORES):
        c0, c1 = k * CPC, (k + 1) * CPC
        # gbtk: d-major transposed gb (own slice for RDMA, full otherwise)
        gsl = gb[c0:c1] if K_RDMA else gb
        gbtk = np.ascontiguousarray(
            gsl.T.reshape(NCH, 128, gsl.shape[0]).transpose(1, 0, 2)
        ).astype(ml_dtypes.bfloat16)
        peers = np.zeros((1, 8), np.int32)
        for d in range(1, 8):
            peers[0, d] = NC_BASE[(k + d) % 8]
        in_maps.append({
            "mem": memory[c0:c1].reshape(R, D),
            "bk": np.ascontiguousarray(inputs["global_bias_key"][c0:c1],
                                       dtype=np.float32).astype(ml_dtypes.bfloat16),
            "bv": np.ascontiguousarray(inputs["global_bias_value"][c0:c1],
                                       dtype=np.float32).astype(ml_dtypes.bfloat16),
            "bffn": np.ascontiguousarray(inputs["global_ffn_bias"][c0:c1],
                                         dtype=np.float32).astype(ml_dtypes.bfloat16),
            "gbtk": gbtk.reshape(128, -1),
            "img": img,
            "imgt": imgt,
            "ls": np.asarray(inputs["logit_scale"], np.float32).reshape(1, 1),
            "ext": consts["ext"],
            "ind": indq,
            "cmask": cmaskb,
            "ident_f": consts["ident"],
            "ident_b": identb,
            "peers": peers,
        })
    return in_maps


def kernel(img_feat, memory, global_bias, global_bias_key, global_bias_value,
           global_ffn_bias, logit_scale, _trace=False):
    nc = _get_nc()
    consts = _host_constants()
    in_maps = _make_in_maps(dict(
        img_feat=img_feat, memory=memory, global_bias=global_bias,
        global_bias_key=global_bias_key, global_bias_value=global_bias_value,
        global_ffn_bias=global_ffn_bias, logit_scale=logit_scale), consts)
    res = run_bass_kernel_spmd(nc, in_maps, core_ids=list(range(N_CORES)),
                               trace=_trace)
    out = np.concatenate([res.results[k]["probs"][:, 0] for k in range(N_CORES)])
    kernel._last_result = res
    return out.reshape(1, C).astype(np.float32)
